# revision 29
# baseline (speedup 1.0000x reference)
"""Llama-style GQA attention (B=1, S=2048, D=4096, 32 q-heads / 8 kv-heads,
rope, causal) on 8 trn2 NeuronCores, tensor-parallel over heads.

Core c owns q-heads 4c..4c+3 and kv-head c. Activations live in
"transposed" (feature-on-partition, seq-on-free) layout so every matmul
contracts over the partition dim. W_O is row-sharded; each core emits a
partial (D, S) fp16 output and the host sums the 8 partials.

v3 schedule: one dense PE pipeline, software-pipelined at the seq-chunk
level: cp0, rope0, cp1, attn0, rope1, cp2, attn1, ... so the rope DVE
chain for chunk j always executes under the next chunk's projection
matmuls, never on the PE critical path.
 - QKV projection per seq-chunk j: all 6 accumulators (4 q-feature tiles
   + k + v) live in PSUM as three 2-bank tiles; weights resident in SBUF
   loaded with 12 batched descriptors (8 k-tiles per DMA), x streams in
   1MB chunks (DMA issue rate was the v2 bottleneck, not bandwidth).
 - PSUM accumulators evacuate through fast ACT copies into SBUF staging,
   freeing banks for the next chunk within ~4us.
 - Scores for 2 k-tiles share one [128,1024] 2-bank PSUM tile -> single
   ACTIVATE exp (amortizes the 352-cycle ACT overhead).
 - Softmax denominator: ones-column matmuls accumulate next to ctx;
   normalization = gpsimd partition-broadcast + reciprocal_approx_fast
   (51 ULP, ~5x faster than DVE reciprocal) + one in-place multiply,
   all off the PE path.
 - O-projection writes fp16 partials (halves the output DMA).

RoPE trick: wq/wk rows are de-interleaved per head on the host
([0,2,..,126,1,3,..,127]) so the on-device pair (2j, 2j+1) becomes
(j, j+64) — a 64-partition block swap done with partition-offset vector
ops against host-precomputed sign-folded cos/sin tables. The permutation
cancels in Q.K, and V/W_O are untouched by it.

Softmax needs no max-subtraction: scores are bounded by construction
(|s| < ~10 => exp safe in fp32). scoresT layout (k on partitions, q on
free) means P feeds P@V with no transpose.
"""
import os
import numpy as np
import ml_dtypes

S = 2048
D = 4096
HD = 128
CH = 512
KT = 32          # contraction tiles over D
NJ = 4           # seq chunks
SCALE = 1.0 / np.sqrt(128.0)

_cache = {}


def _build():
    import concourse.bacc as bacc
    import concourse.tile as tile
    import concourse.mybir as mybir
    from concourse import bass, bass_isa

    dt = mybir.dt
    nc = bacc.Bacc("TRN2", target_bir_lowering=False, debug=False,
                   enable_asserts=False, num_devices=8)

    def inp(name, shape, d):
        return nc.dram_tensor(name, shape, d, kind="ExternalInput").ap()

    # host-pre-tiled inputs: partition-major so every DMA line is >=1KB
    # contiguous (plain feature-major layouts gather 1KB rows at ~85GB/s;
    # these stream at near-full HBM bandwidth)
    xtld = inp("xtld", (HD, NJ, KT, CH), dt.bfloat16)
    wqtld = inp("wqtld", (HD, KT, 4 * HD), dt.bfloat16)
    wktld = inp("wktld", (HD, KT, HD), dt.bfloat16)
    wvtld = inp("wvtld", (HD, KT, HD), dt.bfloat16)
    wotld = inp("wotld", (HD, 32, 4, HD), dt.bfloat16)
    cosd = inp("cosd", (HD, S), dt.float32)
    sind = inp("sind", (HD, S), dt.float32)
    maskd = inp("maskd", (HD, 4 * CH), dt.bfloat16)
    identd = inp("identd", (HD, HD), dt.bfloat16)
    onesc = inp("onesc", (HD, 1), dt.bfloat16)
    outT = nc.dram_tensor("outT", (D, S), dt.float16, kind="ExternalOutput").ap()

    Exp = mybir.ActivationFunctionType.Exp

    with tile.TileContext(nc) as tc:
        with (
            tc.tile_pool(name="const", bufs=1) as constp,
            tc.tile_pool(name="wres", bufs=1) as wresp,
            tc.tile_pool(name="xs", bufs=4) as xpool,
            tc.tile_pool(name="wo", bufs=6) as wopool,
            tc.tile_pool(name="acts", bufs=1) as actp,
            tc.tile_pool(name="pt", bufs=4) as ptpool,
            tc.tile_pool(name="tmp", bufs=2) as tmpp,
            tc.tile_pool(name="ost", bufs=3) as ostp,
            tc.tile_pool(name="ps", bufs=4, space="PSUM") as psp,
        ):

            # resident weights: batched loads; first chunks small so the PE
            # can start ~10us earlier (1KB-line DMA patterns run ~85GB/s)
            CHUNKS = [(0, 2), (2, 6), (8, 8), (16, 8), (24, 8)]  # (k0, nk)
            wq8 = [wresp.tile([HD, nk, 4 * HD], dt.bfloat16, tag=f"wq8_{c}",
                              name=f"wq8_{c}") for c, (k0, nk) in enumerate(CHUNKS)]
            wk8 = [wresp.tile([HD, nk, HD], dt.bfloat16, tag=f"wk8_{c}",
                              name=f"wk8_{c}") for c, (k0, nk) in enumerate(CHUNKS)]
            wv8 = [wresp.tile([HD, nk, HD], dt.bfloat16, tag=f"wv8_{c}",
                              name=f"wv8_{c}") for c, (k0, nk) in enumerate(CHUNKS)]

            def load_wc(c, eng):
                k0, nk = CHUNKS[c]
                eng.dma_start(wq8[c][:], wqtld[:, k0:k0 + nk, :])
                eng.dma_start(wk8[c][:], wktld[:, k0:k0 + nk, :])
                eng.dma_start(wv8[c][:], wvtld[:, k0:k0 + nk, :])

            # weights ride the (otherwise idle) scalar ring: x owns sync,
            # consts + wo + broadcasts own gpsimd
            load_wc(0, nc.scalar)
            onesc_t = constp.tile([HD, 1], dt.bfloat16, tag="onesc")
            nc.gpsimd.dma_start(onesc_t[:], onesc[:])
            ident_t = constp.tile([HD, HD], dt.bfloat16, tag="ident")
            nc.gpsimd.dma_start(ident_t[:], identd[:])
            # warm the ACT exp table while startup DMAs run
            dummy = tmpp.tile([HD, 1], dt.float32, tag="dummy", bufs=1)
            nc.scalar.activation(dummy[:], onesc_t[:], Exp)
            for c in (1, 2):
                load_wc(c, nc.scalar)
            for c in (3, 4):
                load_wc(c, nc.gpsimd)
            cos_t = constp.tile([HD, S], dt.float32, tag="cos")
            nc.gpsimd.dma_start(cos_t[:], cosd[:])
            sin_t = constp.tile([HD, S], dt.float32, tag="sin")
            nc.gpsimd.dma_start(sin_t[:], sind[:])
            mask_t = constp.tile([HD, 4 * CH], dt.bfloat16, tag="mask")
            nc.gpsimd.dma_start(mask_t[:], maskd[:])

            # persistent activations (feature x seq)
            ktr = actp.tile([HD, S], dt.bfloat16, tag="ktr")
            vbuf = actp.tile([HD, S], dt.bfloat16, tag="vbuf")  # (k 128, kt*128 d)
            ctxs = [actp.tile([HD, S], dt.bfloat16, tag=f"ctx{h}", name=f"ctx{h}")
                    for h in range(4)]

            def rope_into(dst, src, ch):
                """dst (bf16 [128,512]) = src*COS + swap64(src)*SIN at chunk ch"""
                c0 = ch * CH
                t1 = tmpp.tile([HD, CH], dt.float32, tag="r1")
                nc.vector.tensor_mul(t1[:], src, cos_t[:, c0:c0 + CH])
                t2 = tmpp.tile([HD, CH], dt.float32, tag="r2")
                nc.vector.tensor_mul(t2[0:64, :], src[64:128, :], sin_t[64:128, c0:c0 + CH])
                nc.vector.tensor_mul(t2[64:128, :], src[0:64, :], sin_t[0:64, c0:c0 + CH])
                nc.vector.tensor_add(dst, t1[:], t2[:])

            qtr = {}

            def chpass(j, mid=None):
                """project x chunk j; stage to SBUF. rope + v-transpose deferred."""
                qA = psp.tile([HD, 2 * CH], dt.float32, tag="b2", name=f"qA{j}")
                qB = psp.tile([HD, 2 * CH], dt.float32, tag="b2", name=f"qB{j}")
                kvA = psp.tile([HD, 2 * CH], dt.float32, tag="b2", name=f"kv{j}")
                for c, (k0, nk) in enumerate(CHUNKS):
                    x8 = xpool.tile([HD, 8, CH], dt.bfloat16, tag="x8",
                                    name=f"x8_{j}_{c}")
                    nc.sync.dma_start(x8[:, 0:nk, :],
                                      xtld[:, j, k0:k0 + nk, :])
                    for kk in range(nk):
                        k = k0 + kk
                        st = (k == 0)
                        sp = (k == KT - 1)
                        xs = x8[:, kk, :]
                        nc.tensor.matmul(qA[:, 0:CH], wq8[c][:, kk, 0:HD], xs, start=st, stop=sp)
                        nc.tensor.matmul(qA[:, CH:2 * CH], wq8[c][:, kk, HD:2 * HD], xs, start=st, stop=sp)
                        nc.tensor.matmul(qB[:, 0:CH], wq8[c][:, kk, 2 * HD:3 * HD], xs, start=st, stop=sp)
                        nc.tensor.matmul(qB[:, CH:2 * CH], wq8[c][:, kk, 3 * HD:4 * HD], xs, start=st, stop=sp)
                        nc.tensor.matmul(kvA[:, 0:CH], wk8[c][:, kk, :], xs, start=st, stop=sp)
                        nc.tensor.matmul(kvA[:, CH:2 * CH], wv8[c][:, kk, :], xs, start=st, stop=sp)
                    if c == 1 and mid is not None:
                        mid()  # prev chunk's v-transposes, off the boundary
                # evacuate PSUM accumulators to SBUF staging, split across the
                # two PSUM-capable engines so attn's first exp starts ~2us sooner
                vstage = tmpp.tile([HD, CH], dt.bfloat16, tag="vstage",
                                   name=f"vstage{j}")
                nc.scalar.copy(vstage[:], kvA[:, CH:2 * CH])
                qsB = tmpp.tile([HD, 2 * CH], dt.float32, tag="qsB", bufs=1, name=f"qsB{j}")
                nc.vector.tensor_copy(qsB[:], qB[:])
                kst = tmpp.tile([HD, CH], dt.float32, tag="kst", name=f"kst{j}")
                nc.vector.tensor_copy(kst[:], kvA[:, 0:CH])
                qsA = tmpp.tile([HD, 2 * CH], dt.float32, tag="qsA", bufs=1, name=f"qsA{j}")
                nc.scalar.copy(qsA[:], qA[:])
                return kst, qsA, qsB, vstage

            def vxpose(j, vstage):
                """v: PE-transpose 4x128 into one psum slot, single copy to vbuf"""
                vtp = psp.tile([HD, 4 * CH], dt.bfloat16, tag="b2", name=f"vtp{j}")
                for t in range(4):
                    nc.tensor.transpose(vtp[:, t * HD:(t + 1) * HD],
                                        vstage[:, t * HD:(t + 1) * HD], ident_t[:])
                nc.vector.tensor_copy(vbuf[:, j * CH:(j + 1) * CH], vtp[:, 0:CH])

            def rope_block(j, kst, qsA, qsB, _vstage=None):
                rope_into(ktr[:, j * CH:(j + 1) * CH], kst[:], j)
                for f in range(4):
                    qt = ptpool.tile([HD, CH], dt.bfloat16, tag="qtr", bufs=8,
                                     name=f"qtr{j}_{f}")
                    src = qsA if f < 2 else qsB
                    rope_into(qt[:], src[:, (f % 2) * CH:(f % 2 + 1) * CH], j)
                    qtr[(j, f)] = qt

            def attn(j):
                for h in range(4):
                    ng = 2 * (j + 1)
                    ctxden = psp.tile([HD, 2 * CH], dt.float32, tag="b2",
                                      name=f"cd{j}_{h}")
                    # zero the den bank so col-tiled partial rows can skip
                    # regions the ragged matmuls never write
                    nc.vector.memset(ctxden[:, CH:2 * CH], 0)
                    pts = []

                    def score_exp(g):
                        sg = psp.tile([HD, 2 * CH], dt.float32, tag="b2",
                                      name=f"sg{j}_{h}_{g}")
                        pt = ptpool.tile([HD, 2 * CH], dt.bfloat16, tag="pt",
                                         bufs=5, name=f"pt{j}_{h}_{g}")
                        if g < 2 * j:  # full group: batched exp, no mask
                            nc.tensor.matmul(sg[:, 0:CH],
                                             ktr[:, (2 * g) * HD:(2 * g + 1) * HD],
                                             qtr[(j, h)][:], start=True, stop=True)
                            nc.tensor.matmul(sg[:, CH:2 * CH],
                                             ktr[:, (2 * g + 1) * HD:(2 * g + 2) * HD],
                                             qtr[(j, h)][:], start=True, stop=True)
                            nc.scalar.activation(pt[:], sg[:], Exp, scale=SCALE)
                        else:  # diagonal: ragged live region + [128,128] tri mask
                            for half in range(2):
                                kt = 2 * g + half
                                q0 = (kt - 4 * j) * HD
                                c0, c1 = half * CH + q0, (half + 1) * CH
                                nc.tensor.matmul(sg[:, c0:c1],
                                                 ktr[:, kt * HD:(kt + 1) * HD],
                                                 qtr[(j, h)][:, q0:CH],
                                                 start=True, stop=True)
                                nc.scalar.activation(pt[:, c0:c1], sg[:, c0:c1],
                                                     Exp, scale=SCALE)
                                nc.vector.tensor_mul(pt[:, c0:c0 + HD],
                                                     pt[:, c0:c0 + HD],
                                                     mask_t[:, 0:HD])
                        pts.append(pt)

                    def pv_den2(g0):
                        """PV + den for group pair (g0, g0+1); the 4 den
                        matmuls target 4 distinct PE column groups (partitions
                        0/32/64/96) so they execute ~concurrently."""
                        quads = []
                        for g in (g0, g0 + 1):
                            pt = pts[g]
                            for half in range(2):
                                kt = 2 * g + half
                                q0 = max(0, (kt - 4 * j)) * HD
                                quads.append((pt, half, kt, q0))
                        for pt, half, kt, q0 in quads:
                            nc.tensor.matmul(ctxden[:, q0:CH],
                                             vbuf[:, kt * HD:(kt + 1) * HD],
                                             pt[:, half * CH + q0:(half + 1) * CH],
                                             start=(kt == 0),
                                             stop=(kt == 2 * ng - 1))
                        plain = (j == NJ - 1 and h == 3)
                        for pt, half, kt, q0 in quads:
                            m = 0 if plain else kt % 4
                            nc.tensor.matmul(ctxden[32 * m:32 * m + 1,
                                                    CH + q0:2 * CH], onesc_t[:],
                                             pt[:, half * CH + q0:(half + 1) * CH],
                                             start=(kt == 0),
                                             stop=(kt == 2 * ng - 1) if plain
                                             else (kt >= 2 * ng - 4),
                                             tile_position=(0, 32 * m),
                                             skip_group_check=True)

                    LAG = 3
                    for gi in range(ng + LAG + 1):
                        if gi < ng:
                            score_exp(gi)
                        if gi >= LAG and (gi - LAG) % 2 == 1:
                            pv_den2(gi - LAG - 1)
                    # evacuate raw ctx via ACT (idle here; frees bank fast)
                    nc.scalar.copy(ctxs[h][:, j * CH:(j + 1) * CH],
                                   ctxden[:, 0:CH])
                    if j == NJ - 1 and h == 3:
                        # last head: minimal-latency chain so O-proj's first
                        # cf=3 matmuls aren't gated (~3us shorter than PAR)
                        dsb = tmpp.tile([1, CH], dt.float32, tag="dsb", bufs=1,
                                        name=f"dsb{j}_{h}")
                        nc.vector.tensor_copy(dsb[:], ctxden[0:1, CH:2 * CH])
                        rcp1 = tmpp.tile([1, CH], dt.float32, tag="rcp1", bufs=1,
                                         name=f"rcp1_{j}_{h}")
                        nc.vector.reciprocal_approx_fast(rcp1[:], dsb[:])
                        bcs = tmpp.tile([HD, CH], dt.float32, tag="bcs",
                                        name=f"bcs{j}_{h}")
                        nc.gpsimd.partition_broadcast(bcs[:], rcp1[:], channels=HD)
                    else:
                        # den = sum over all 128 partitions (memset zeroed the
                        # unused rows) -> one gpsimd all-reduce, pre-broadcast;
                        # one DVE copy frees the PSUM bank immediately
                        dsf = tmpp.tile([HD, CH], dt.float32, tag="dsf",
                                        name=f"dsf{j}_{h}")
                        nc.vector.tensor_copy(dsf[:], ctxden[:, CH:2 * CH])
                        bcs = tmpp.tile([HD, CH], dt.float32, tag="bcs",
                                        name=f"bcs{j}_{h}")
                        nc.gpsimd.partition_all_reduce(
                            bcs[:], dsf[:], channels=HD,
                            reduce_op=bass_isa.ReduceOp.add)
                        rcp = tmpp.tile([HD, CH], dt.float32, tag="rcp",
                                        name=f"rcp{j}_{h}")
                        nc.vector.reciprocal_approx_fast(rcp[:], bcs[:])
                        bcs = rcp
                    nc.vector.tensor_mul(ctxs[h][:, j * CH:(j + 1) * CH],
                                         ctxs[h][:, j * CH:(j + 1) * CH], bcs[:])

            # software pipeline: rope(j) always runs under cp(j+1)/attn PE work;
            # vxpose(j) is emitted inside cp(j+1)'s k-loop, off the boundary
            staged = {0: chpass(0)}
            rope_block(0, *staged[0])
            for j in range(NJ):
                if j + 1 < NJ:
                    staged[j + 1] = chpass(
                        j + 1, mid=lambda vj=j: vxpose(vj, staged[vj][3]))
                attn(j)
                if j + 1 < NJ:
                    rope_block(j + 1, *staged[j + 1])
                if j == NJ - 2:
                    vxpose(NJ - 1, staged[NJ - 1][3])
                    # prefetch the first O-proj weights ahead of attn(3)'s
                    # partition_all_reduces on the gpsimd queue
                    wo_pre = []
                    for of in range(4):
                        w = wopool.tile([HD, 4, HD], dt.bfloat16, tag="wo",
                                        name=f"wopre{of}")
                        nc.gpsimd.dma_start(w[:], wotld[:, of, :, :])
                        wo_pre.append(w)

            # ---- O projection (row-sharded W_O -> partial fp16 outT) ----
            for of in range(32):
                oA = psp.tile([HD, 2 * CH], dt.float32, tag="b2", name=f"oA{of}")
                oB = psp.tile([HD, 2 * CH], dt.float32, tag="b2", name=f"oB{of}")
                if of < 4:
                    wot8 = wo_pre[of]
                else:
                    wot8 = wopool.tile([HD, 4, HD], dt.bfloat16, tag="wo")
                    nc.gpsimd.dma_start(wot8[:], wotld[:, of, :, :])
                for cf in range(4):
                    st = (cf == 0)
                    sp = (cf == 3)
                    for ch in range(4):
                        dstp = oA if ch < 2 else oB
                        nc.tensor.matmul(dstp[:, (ch % 2) * CH:(ch % 2 + 1) * CH],
                                         wot8[:, cf, :],
                                         ctxs[cf][:, ch * CH:(ch + 1) * CH],
                                         start=st, stop=sp)
                ost1 = ostp.tile([HD, 2 * CH], dt.float16, tag="ostA")
                ost2 = ostp.tile([HD, 2 * CH], dt.float16, tag="ostB")
                if of < 31:
                    nc.vector.tensor_copy(ost1[:], oA[:])
                    nc.scalar.copy(ost2[:], oB[:])
                    nc.sync.dma_start(outT[of * HD:(of + 1) * HD, 0:2 * CH], ost1[:])
                    nc.sync.dma_start(outT[of * HD:(of + 1) * HD, 2 * CH:4 * CH], ost2[:])
                else:  # drain the tail fast: split work across engines/queues
                    nc.vector.tensor_copy(ost1[:, 0:CH], oA[:, 0:CH])
                    nc.scalar.copy(ost2[:, 0:CH], oB[:, 0:CH])
                    nc.sync.dma_start(outT[of * HD:(of + 1) * HD, 0:CH], ost1[:, 0:CH])
                    nc.gpsimd.dma_start(outT[of * HD:(of + 1) * HD, 2 * CH:3 * CH], ost2[:, 0:CH])
                    nc.vector.tensor_copy(ost1[:, CH:2 * CH], oA[:, CH:2 * CH])
                    nc.scalar.copy(ost2[:, CH:2 * CH], oB[:, CH:2 * CH])
                    nc.sync.dma_start(outT[of * HD:(of + 1) * HD, CH:2 * CH], ost1[:, CH:2 * CH])
                    nc.gpsimd.dma_start(outT[of * HD:(of + 1) * HD, 3 * CH:4 * CH], ost2[:, CH:2 * CH])

    nc.compile()
    return nc


def _host_inputs(x, wq, wk, wv, wo):
    bf16 = ml_dtypes.bfloat16
    perm = np.concatenate([np.arange(0, 128, 2), np.arange(1, 128, 2)])
    half = 64
    inv = 1.0 / (10000.0 ** (np.arange(half) / half))
    ang = np.arange(S)[:, None] * inv[None, :]
    cosd = np.ascontiguousarray(
        np.concatenate([np.cos(ang).T, np.cos(ang).T], 0)).astype(np.float32)
    sind = np.ascontiguousarray(
        np.concatenate([np.sin(ang).T, -np.sin(ang).T], 0)).astype(np.float32)
    maskd = np.zeros((HD, 4 * CH), np.float32)
    for m in range(4):
        kl = np.arange(HD)[:, None]
        maskd[:, m * CH:(m + 1) * CH] = (np.arange(CH)[None, :] >= HD * m + kl)
    maskd = maskd.astype(bf16)
    identd = np.eye(HD, dtype=bf16)
    onescol = np.ones((HD, 1), bf16)
    xTb = np.ascontiguousarray(x[0].T).astype(bf16)

    def tile_pkn(wT):  # (D, n) feature-major -> (128, 32k, n) partition-major
        n = wT.shape[1]
        return np.ascontiguousarray(
            wT.reshape(KT, HD, n).transpose(1, 0, 2)).astype(bf16)

    xtld = np.ascontiguousarray(
        xTb.reshape(KT, HD, NJ, CH).transpose(1, 2, 0, 3))  # (128, j, k, n)

    in_maps = []
    for c in range(8):
        qrows = slice(512 * c, 512 * (c + 1))
        wq_c = wq[qrows].reshape(4, HD, D)[:, perm].reshape(512, D)
        wk_c = wk[HD * c:HD * (c + 1)][perm]
        wv_c = wv[HD * c:HD * (c + 1)]
        wo_c = np.ascontiguousarray(wo[:, qrows].T)  # (512, 4096)
        wotld = np.ascontiguousarray(
            wo_c.reshape(4, HD, 32, HD).transpose(1, 2, 0, 3)).astype(bf16)
        in_maps.append({
            "xtld": xtld,
            "wqtld": tile_pkn(np.ascontiguousarray(wq_c.T)),
            "wktld": tile_pkn(np.ascontiguousarray(wk_c.T)),
            "wvtld": tile_pkn(np.ascontiguousarray(wv_c.T)),
            "wotld": wotld,
            "cosd": cosd, "sind": sind, "maskd": maskd, "identd": identd,
            "onesc": onescol,
        })
    return in_maps


LAST_RESULTS = None


def kernel(x, wq, wk, wv, wo, attn_mask):
    global LAST_RESULTS
    from concourse import bass_utils
    if "nc" not in _cache:
        _cache["nc"] = _build()
    nc = _cache["nc"]
    in_maps = _host_inputs(np.asarray(x, np.float32), np.asarray(wq, np.float32),
                           np.asarray(wk, np.float32), np.asarray(wv, np.float32),
                           np.asarray(wo, np.float32))
    res = bass_utils.run_bass_kernel_spmd(
        nc, in_maps, core_ids=list(range(8)),
        trace=bool(os.environ.get("BASS_TRACE")))
    LAST_RESULTS = res
    acc = res.results[0]["outT"].astype(np.float64)
    for c in range(1, 8):
        acc = acc + res.results[c]["outT"]
    return np.ascontiguousarray(acc.T).astype(np.float32).reshape(1, S, D)


# revision 30
# speedup vs baseline: 1.0480x; 1.0480x over previous
"""Llama-style GQA attention (B=1, S=2048, D=4096, 32 q-heads / 8 kv-heads,
rope, causal) on 8 trn2 NeuronCores, tensor-parallel over heads.

Core c owns q-heads 4c..4c+3 and kv-head c. Activations live in
"transposed" (feature-on-partition, seq-on-free) layout so every matmul
contracts over the partition dim. W_O is row-sharded; each core emits a
partial (D, S) fp16 output and the host sums the 8 partials.

v3 schedule: one dense PE pipeline, software-pipelined at the seq-chunk
level: cp0, rope0, cp1, attn0, rope1, cp2, attn1, ... so the rope DVE
chain for chunk j always executes under the next chunk's projection
matmuls, never on the PE critical path.
 - QKV projection per seq-chunk j: all 6 accumulators (4 q-feature tiles
   + k + v) live in PSUM as three 2-bank tiles; weights resident in SBUF
   loaded with 12 batched descriptors (8 k-tiles per DMA), x streams in
   1MB chunks (DMA issue rate was the v2 bottleneck, not bandwidth).
 - PSUM accumulators evacuate through fast ACT copies into SBUF staging,
   freeing banks for the next chunk within ~4us.
 - Scores for 2 k-tiles share one [128,1024] 2-bank PSUM tile -> single
   ACTIVATE exp (amortizes the 352-cycle ACT overhead).
 - Softmax denominator: ones-column matmuls accumulate next to ctx;
   normalization = gpsimd partition-broadcast + reciprocal_approx_fast
   (51 ULP, ~5x faster than DVE reciprocal) + one in-place multiply,
   all off the PE path.
 - O-projection writes fp16 partials (halves the output DMA).

RoPE trick: wq/wk rows are de-interleaved per head on the host
([0,2,..,126,1,3,..,127]) so the on-device pair (2j, 2j+1) becomes
(j, j+64) — a 64-partition block swap done with partition-offset vector
ops against host-precomputed sign-folded cos/sin tables. The permutation
cancels in Q.K, and V/W_O are untouched by it.

Softmax needs no max-subtraction: scores are bounded by construction
(|s| < ~10 => exp safe in fp32). scoresT layout (k on partitions, q on
free) means P feeds P@V with no transpose.
"""
import os
import numpy as np
import ml_dtypes

S = 2048
D = 4096
HD = 128
CH = 512
KT = 32          # contraction tiles over D
NJ = 4           # seq chunks
SCALE = 1.0 / np.sqrt(128.0)

_cache = {}


def _build():
    import concourse.bacc as bacc
    import concourse.tile as tile
    import concourse.mybir as mybir
    from concourse import bass, bass_isa

    dt = mybir.dt
    nc = bacc.Bacc("TRN2", target_bir_lowering=False, debug=False,
                   enable_asserts=False, num_devices=8)

    def inp(name, shape, d):
        return nc.dram_tensor(name, shape, d, kind="ExternalInput").ap()

    # host-pre-tiled inputs: partition-major so every DMA line is >=1KB
    # contiguous (plain feature-major layouts gather 1KB rows at ~85GB/s;
    # these stream at near-full HBM bandwidth)
    xtld = inp("xtld", (HD, NJ, KT, CH), dt.bfloat16)
    wqtld = inp("wqtld", (HD, KT, 4 * HD), dt.bfloat16)
    wktld = inp("wktld", (HD, KT, HD), dt.bfloat16)
    wvtld = inp("wvtld", (HD, KT, HD), dt.bfloat16)
    wotld = inp("wotld", (HD, 32, 4, HD), dt.bfloat16)
    cosd = inp("cosd", (HD, S), dt.float32)
    sind = inp("sind", (HD, S), dt.float32)
    maskd = inp("maskd", (HD, 4 * CH), dt.bfloat16)
    identd = inp("identd", (HD, HD), dt.bfloat16)
    onesc = inp("onesc", (HD, 1), dt.bfloat16)
    outT = nc.dram_tensor("outT", (D, S), dt.float16, kind="ExternalOutput").ap()

    Exp = mybir.ActivationFunctionType.Exp

    with tile.TileContext(nc) as tc:
        with (
            tc.tile_pool(name="const", bufs=1) as constp,
            tc.tile_pool(name="wres", bufs=1) as wresp,
            tc.tile_pool(name="xs", bufs=8) as xpool,
            tc.tile_pool(name="wo", bufs=6) as wopool,
            tc.tile_pool(name="acts", bufs=1) as actp,
            tc.tile_pool(name="pt", bufs=4) as ptpool,
            tc.tile_pool(name="tmp", bufs=2) as tmpp,
            tc.tile_pool(name="ost", bufs=3) as ostp,
            tc.tile_pool(name="ps", bufs=4, space="PSUM") as psp,
        ):

            # resident weights. Each DMA ring sustains only ~70-90GB/s, so
            # chunk finely (4 k-tiles) and round-robin rings by need-time.
            NC_ = 8
            CHUNKS = [(4 * i, 4) for i in range(NC_)]
            wq8 = [wresp.tile([HD, 4, 4 * HD], dt.bfloat16, tag=f"wq8_{c}",
                              name=f"wq8_{c}") for c in range(NC_)]
            wk8 = [wresp.tile([HD, 4, HD], dt.bfloat16, tag=f"wk8_{c}",
                              name=f"wk8_{c}") for c in range(NC_)]
            wv8 = [wresp.tile([HD, 4, HD], dt.bfloat16, tag=f"wv8_{c}",
                              name=f"wv8_{c}") for c in range(NC_)]

            def load_wc(c, eng):
                k0, nk = CHUNKS[c]
                eng.dma_start(wq8[c][:], wqtld[:, k0:k0 + nk, :])
                eng.dma_start(wk8[c][:], wktld[:, k0:k0 + nk, :])
                eng.dma_start(wv8[c][:], wvtld[:, k0:k0 + nk, :])

            load_wc(0, nc.scalar)
            onesc_t = constp.tile([HD, 1], dt.bfloat16, tag="onesc")
            nc.gpsimd.dma_start(onesc_t[:], onesc[:])
            ident_t = constp.tile([HD, HD], dt.bfloat16, tag="ident")
            nc.gpsimd.dma_start(ident_t[:], identd[:])
            # warm the ACT exp table while startup DMAs run
            dummy = tmpp.tile([HD, 1], dt.float32, tag="dummy", bufs=1)
            nc.scalar.activation(dummy[:], onesc_t[:], Exp)
            for c in range(1, NC_):
                load_wc(c, nc.scalar if c % 2 == 0 else nc.gpsimd)
            cos_t = constp.tile([HD, S], dt.float32, tag="cos")
            nc.scalar.dma_start(cos_t[:], cosd[:])
            sin_t = constp.tile([HD, S], dt.float32, tag="sin")
            nc.gpsimd.dma_start(sin_t[:], sind[:])
            mask_t = constp.tile([HD, 4 * CH], dt.bfloat16, tag="mask")
            nc.gpsimd.dma_start(mask_t[:], maskd[:])

            # persistent activations (feature x seq)
            ktr = actp.tile([HD, S], dt.bfloat16, tag="ktr")
            vbuf = actp.tile([HD, S], dt.bfloat16, tag="vbuf")  # (k 128, kt*128 d)
            ctxs = [actp.tile([HD, S], dt.bfloat16, tag=f"ctx{h}", name=f"ctx{h}")
                    for h in range(4)]

            def rope_into(dst, src, ch):
                """dst (bf16 [128,512]) = src*COS + swap64(src)*SIN at chunk ch"""
                c0 = ch * CH
                t1 = tmpp.tile([HD, CH], dt.float32, tag="r1")
                nc.vector.tensor_mul(t1[:], src, cos_t[:, c0:c0 + CH])
                t2 = tmpp.tile([HD, CH], dt.float32, tag="r2")
                nc.vector.tensor_mul(t2[0:64, :], src[64:128, :], sin_t[64:128, c0:c0 + CH])
                nc.vector.tensor_mul(t2[64:128, :], src[0:64, :], sin_t[0:64, c0:c0 + CH])
                nc.vector.tensor_add(dst, t1[:], t2[:])

            qtr = {}

            def chpass(j, mid=None):
                """project x chunk j; stage to SBUF. rope + v-transpose deferred."""
                qA = psp.tile([HD, 2 * CH], dt.float32, tag="b2", name=f"qA{j}")
                qB = psp.tile([HD, 2 * CH], dt.float32, tag="b2", name=f"qB{j}")
                kvA = psp.tile([HD, 2 * CH], dt.float32, tag="b2", name=f"kv{j}")
                for c, (k0, nk) in enumerate(CHUNKS):
                    x8 = xpool.tile([HD, 4, CH], dt.bfloat16, tag="x8",
                                    name=f"x8_{j}_{c}")
                    if j == 0 or c % 2 == 0:
                        xeng = nc.sync
                    else:  # issued ahead of attn(j-1)'s exps on this queue
                        xeng = nc.scalar
                    xeng.dma_start(x8[:, 0:nk, :],
                                   xtld[:, j, k0:k0 + nk, :])
                    for kk in range(nk):
                        k = k0 + kk
                        st = (k == 0)
                        sp = (k == KT - 1)
                        xs = x8[:, kk, :]
                        nc.tensor.matmul(qA[:, 0:CH], wq8[c][:, kk, 0:HD], xs, start=st, stop=sp)
                        nc.tensor.matmul(qA[:, CH:2 * CH], wq8[c][:, kk, HD:2 * HD], xs, start=st, stop=sp)
                        nc.tensor.matmul(qB[:, 0:CH], wq8[c][:, kk, 2 * HD:3 * HD], xs, start=st, stop=sp)
                        nc.tensor.matmul(qB[:, CH:2 * CH], wq8[c][:, kk, 3 * HD:4 * HD], xs, start=st, stop=sp)
                        nc.tensor.matmul(kvA[:, 0:CH], wk8[c][:, kk, :], xs, start=st, stop=sp)
                        nc.tensor.matmul(kvA[:, CH:2 * CH], wv8[c][:, kk, :], xs, start=st, stop=sp)
                    if c == 1 and mid is not None:
                        mid()  # prev chunk's v-transposes, off the boundary
                # evacuate PSUM accumulators to SBUF staging, split across the
                # two PSUM-capable engines so attn's first exp starts ~2us sooner
                vstage = tmpp.tile([HD, CH], dt.bfloat16, tag="vstage",
                                   name=f"vstage{j}")
                nc.scalar.copy(vstage[:], kvA[:, CH:2 * CH])
                qsB = tmpp.tile([HD, 2 * CH], dt.float32, tag="qsB", bufs=1, name=f"qsB{j}")
                nc.vector.tensor_copy(qsB[:], qB[:])
                kst = tmpp.tile([HD, CH], dt.float32, tag="kst", name=f"kst{j}")
                nc.vector.tensor_copy(kst[:], kvA[:, 0:CH])
                qsA = tmpp.tile([HD, 2 * CH], dt.float32, tag="qsA", bufs=1, name=f"qsA{j}")
                nc.scalar.copy(qsA[:], qA[:])
                return kst, qsA, qsB, vstage

            def vxpose(j, vstage):
                """v: PE-transpose 4x128 into one psum slot, single copy to vbuf"""
                vtp = psp.tile([HD, 4 * CH], dt.bfloat16, tag="b2", name=f"vtp{j}")
                for t in range(4):
                    nc.tensor.transpose(vtp[:, t * HD:(t + 1) * HD],
                                        vstage[:, t * HD:(t + 1) * HD], ident_t[:])
                nc.vector.tensor_copy(vbuf[:, j * CH:(j + 1) * CH], vtp[:, 0:CH])

            def rope_block(j, kst, qsA, qsB, _vstage=None):
                rope_into(ktr[:, j * CH:(j + 1) * CH], kst[:], j)
                for f in range(4):
                    qt = ptpool.tile([HD, CH], dt.bfloat16, tag="qtr", bufs=8,
                                     name=f"qtr{j}_{f}")
                    src = qsA if f < 2 else qsB
                    rope_into(qt[:], src[:, (f % 2) * CH:(f % 2 + 1) * CH], j)
                    qtr[(j, f)] = qt

            def attn(j):
                for h in range(4):
                    ng = 2 * (j + 1)
                    ctxden = psp.tile([HD, 2 * CH], dt.float32, tag="b2",
                                      name=f"cd{j}_{h}")
                    # zero the den bank so col-tiled partial rows can skip
                    # regions the ragged matmuls never write
                    nc.vector.memset(ctxden[:, CH:2 * CH], 0)
                    pts = []

                    def score_exp(g):
                        sg = psp.tile([HD, 2 * CH], dt.float32, tag="b2",
                                      name=f"sg{j}_{h}_{g}")
                        pt = ptpool.tile([HD, 2 * CH], dt.bfloat16, tag="pt",
                                         bufs=5, name=f"pt{j}_{h}_{g}")
                        if g < 2 * j:  # full group: batched exp, no mask
                            nc.tensor.matmul(sg[:, 0:CH],
                                             ktr[:, (2 * g) * HD:(2 * g + 1) * HD],
                                             qtr[(j, h)][:], start=True, stop=True)
                            nc.tensor.matmul(sg[:, CH:2 * CH],
                                             ktr[:, (2 * g + 1) * HD:(2 * g + 2) * HD],
                                             qtr[(j, h)][:], start=True, stop=True)
                            nc.scalar.activation(pt[:], sg[:], Exp, scale=SCALE)
                        else:  # diagonal: ragged live region + [128,128] tri mask
                            for half in range(2):
                                kt = 2 * g + half
                                q0 = (kt - 4 * j) * HD
                                c0, c1 = half * CH + q0, (half + 1) * CH
                                nc.tensor.matmul(sg[:, c0:c1],
                                                 ktr[:, kt * HD:(kt + 1) * HD],
                                                 qtr[(j, h)][:, q0:CH],
                                                 start=True, stop=True)
                                nc.scalar.activation(pt[:, c0:c1], sg[:, c0:c1],
                                                     Exp, scale=SCALE)
                                nc.vector.tensor_mul(pt[:, c0:c0 + HD],
                                                     pt[:, c0:c0 + HD],
                                                     mask_t[:, 0:HD])
                        pts.append(pt)

                    def pv_den2(g0):
                        """PV + den for group pair (g0, g0+1); the 4 den
                        matmuls target 4 distinct PE column groups (partitions
                        0/32/64/96) so they execute ~concurrently."""
                        quads = []
                        for g in (g0, g0 + 1):
                            pt = pts[g]
                            for half in range(2):
                                kt = 2 * g + half
                                q0 = max(0, (kt - 4 * j)) * HD
                                quads.append((pt, half, kt, q0))
                        for pt, half, kt, q0 in quads:
                            nc.tensor.matmul(ctxden[:, q0:CH],
                                             vbuf[:, kt * HD:(kt + 1) * HD],
                                             pt[:, half * CH + q0:(half + 1) * CH],
                                             start=(kt == 0),
                                             stop=(kt == 2 * ng - 1))
                        plain = (j == NJ - 1 and h == 3)
                        for pt, half, kt, q0 in quads:
                            m = 0 if plain else kt % 4
                            nc.tensor.matmul(ctxden[32 * m:32 * m + 1,
                                                    CH + q0:2 * CH], onesc_t[:],
                                             pt[:, half * CH + q0:(half + 1) * CH],
                                             start=(kt == 0),
                                             stop=(kt == 2 * ng - 1) if plain
                                             else (kt >= 2 * ng - 4),
                                             tile_position=(0, 32 * m),
                                             skip_group_check=True)

                    LAG = 3
                    for gi in range(ng + LAG + 1):
                        if gi < ng:
                            score_exp(gi)
                        if gi >= LAG and (gi - LAG) % 2 == 1:
                            pv_den2(gi - LAG - 1)
                    # evacuate raw ctx via ACT (idle here; frees bank fast)
                    nc.scalar.copy(ctxs[h][:, j * CH:(j + 1) * CH],
                                   ctxden[:, 0:CH])
                    if j == NJ - 1 and h == 3:
                        # last head: minimal-latency chain so O-proj's first
                        # cf=3 matmuls aren't gated (~3us shorter than PAR)
                        dsb = tmpp.tile([1, CH], dt.float32, tag="dsb", bufs=1,
                                        name=f"dsb{j}_{h}")
                        nc.vector.tensor_copy(dsb[:], ctxden[0:1, CH:2 * CH])
                        rcp1 = tmpp.tile([1, CH], dt.float32, tag="rcp1", bufs=1,
                                         name=f"rcp1_{j}_{h}")
                        nc.vector.reciprocal_approx_fast(rcp1[:], dsb[:])
                        bcs = tmpp.tile([HD, CH], dt.float32, tag="bcs",
                                        name=f"bcs{j}_{h}")
                        nc.gpsimd.partition_broadcast(bcs[:], rcp1[:], channels=HD)
                    else:
                        # den = sum over all 128 partitions (memset zeroed the
                        # unused rows) -> one gpsimd all-reduce, pre-broadcast;
                        # one DVE copy frees the PSUM bank immediately
                        dsf = tmpp.tile([HD, CH], dt.float32, tag="dsf",
                                        name=f"dsf{j}_{h}")
                        nc.vector.tensor_copy(dsf[:], ctxden[:, CH:2 * CH])
                        bcs = tmpp.tile([HD, CH], dt.float32, tag="bcs",
                                        name=f"bcs{j}_{h}")
                        nc.gpsimd.partition_all_reduce(
                            bcs[:], dsf[:], channels=HD,
                            reduce_op=bass_isa.ReduceOp.add)
                        rcp = tmpp.tile([HD, CH], dt.float32, tag="rcp",
                                        name=f"rcp{j}_{h}")
                        nc.vector.reciprocal_approx_fast(rcp[:], bcs[:])
                        bcs = rcp
                    nc.vector.tensor_mul(ctxs[h][:, j * CH:(j + 1) * CH],
                                         ctxs[h][:, j * CH:(j + 1) * CH], bcs[:])

            # software pipeline: rope(j) always runs under cp(j+1)/attn PE work;
            # vxpose(j) is emitted inside cp(j+1)'s k-loop, off the boundary
            staged = {0: chpass(0)}
            rope_block(0, *staged[0])
            for j in range(NJ):
                if j + 1 < NJ:
                    staged[j + 1] = chpass(
                        j + 1, mid=lambda vj=j: vxpose(vj, staged[vj][3]))
                attn(j)
                if j + 1 < NJ:
                    rope_block(j + 1, *staged[j + 1])
                if j == NJ - 2:
                    vxpose(NJ - 1, staged[NJ - 1][3])
                    # prefetch the first O-proj weights ahead of attn(3)'s
                    # partition_all_reduces on the gpsimd queue
                    wo_pre = []
                    for of in range(4):
                        w = wopool.tile([HD, 4, HD], dt.bfloat16, tag="wo",
                                        name=f"wopre{of}")
                        nc.gpsimd.dma_start(w[:], wotld[:, of, :, :])
                        wo_pre.append(w)

            # ---- O projection (row-sharded W_O -> partial fp16 outT) ----
            for of in range(32):
                oA = psp.tile([HD, 2 * CH], dt.float32, tag="b2", name=f"oA{of}")
                oB = psp.tile([HD, 2 * CH], dt.float32, tag="b2", name=f"oB{of}")
                if of < 4:
                    wot8 = wo_pre[of]
                else:
                    wot8 = wopool.tile([HD, 4, HD], dt.bfloat16, tag="wo")
                    nc.gpsimd.dma_start(wot8[:], wotld[:, of, :, :])
                for cf in range(4):
                    st = (cf == 0)
                    sp = (cf == 3)
                    for ch in range(4):
                        dstp = oA if ch < 2 else oB
                        nc.tensor.matmul(dstp[:, (ch % 2) * CH:(ch % 2 + 1) * CH],
                                         wot8[:, cf, :],
                                         ctxs[cf][:, ch * CH:(ch + 1) * CH],
                                         start=st, stop=sp)
                ost1 = ostp.tile([HD, 2 * CH], dt.float16, tag="ostA")
                ost2 = ostp.tile([HD, 2 * CH], dt.float16, tag="ostB")
                if of < 31:
                    nc.vector.tensor_copy(ost1[:], oA[:])
                    nc.scalar.copy(ost2[:], oB[:])
                    nc.sync.dma_start(outT[of * HD:(of + 1) * HD, 0:2 * CH], ost1[:])
                    nc.sync.dma_start(outT[of * HD:(of + 1) * HD, 2 * CH:4 * CH], ost2[:])
                else:  # drain the tail fast: split work across engines/queues
                    nc.vector.tensor_copy(ost1[:, 0:CH], oA[:, 0:CH])
                    nc.scalar.copy(ost2[:, 0:CH], oB[:, 0:CH])
                    nc.sync.dma_start(outT[of * HD:(of + 1) * HD, 0:CH], ost1[:, 0:CH])
                    nc.gpsimd.dma_start(outT[of * HD:(of + 1) * HD, 2 * CH:3 * CH], ost2[:, 0:CH])
                    nc.vector.tensor_copy(ost1[:, CH:2 * CH], oA[:, CH:2 * CH])
                    nc.scalar.copy(ost2[:, CH:2 * CH], oB[:, CH:2 * CH])
                    nc.sync.dma_start(outT[of * HD:(of + 1) * HD, CH:2 * CH], ost1[:, CH:2 * CH])
                    nc.gpsimd.dma_start(outT[of * HD:(of + 1) * HD, 3 * CH:4 * CH], ost2[:, CH:2 * CH])

    nc.compile()
    return nc


def _host_inputs(x, wq, wk, wv, wo):
    bf16 = ml_dtypes.bfloat16
    perm = np.concatenate([np.arange(0, 128, 2), np.arange(1, 128, 2)])
    half = 64
    inv = 1.0 / (10000.0 ** (np.arange(half) / half))
    ang = np.arange(S)[:, None] * inv[None, :]
    cosd = np.ascontiguousarray(
        np.concatenate([np.cos(ang).T, np.cos(ang).T], 0)).astype(np.float32)
    sind = np.ascontiguousarray(
        np.concatenate([np.sin(ang).T, -np.sin(ang).T], 0)).astype(np.float32)
    maskd = np.zeros((HD, 4 * CH), np.float32)
    for m in range(4):
        kl = np.arange(HD)[:, None]
        maskd[:, m * CH:(m + 1) * CH] = (np.arange(CH)[None, :] >= HD * m + kl)
    maskd = maskd.astype(bf16)
    identd = np.eye(HD, dtype=bf16)
    onescol = np.ones((HD, 1), bf16)
    xTb = np.ascontiguousarray(x[0].T).astype(bf16)

    def tile_pkn(wT):  # (D, n) feature-major -> (128, 32k, n) partition-major
        n = wT.shape[1]
        return np.ascontiguousarray(
            wT.reshape(KT, HD, n).transpose(1, 0, 2)).astype(bf16)

    xtld = np.ascontiguousarray(
        xTb.reshape(KT, HD, NJ, CH).transpose(1, 2, 0, 3))  # (128, j, k, n)

    in_maps = []
    for c in range(8):
        qrows = slice(512 * c, 512 * (c + 1))
        wq_c = wq[qrows].reshape(4, HD, D)[:, perm].reshape(512, D)
        wk_c = wk[HD * c:HD * (c + 1)][perm]
        wv_c = wv[HD * c:HD * (c + 1)]
        wo_c = np.ascontiguousarray(wo[:, qrows].T)  # (512, 4096)
        wotld = np.ascontiguousarray(
            wo_c.reshape(4, HD, 32, HD).transpose(1, 2, 0, 3)).astype(bf16)
        in_maps.append({
            "xtld": xtld,
            "wqtld": tile_pkn(np.ascontiguousarray(wq_c.T)),
            "wktld": tile_pkn(np.ascontiguousarray(wk_c.T)),
            "wvtld": tile_pkn(np.ascontiguousarray(wv_c.T)),
            "wotld": wotld,
            "cosd": cosd, "sind": sind, "maskd": maskd, "identd": identd,
            "onesc": onescol,
        })
    return in_maps


LAST_RESULTS = None


def kernel(x, wq, wk, wv, wo, attn_mask):
    global LAST_RESULTS
    from concourse import bass_utils
    if "nc" not in _cache:
        _cache["nc"] = _build()
    nc = _cache["nc"]
    in_maps = _host_inputs(np.asarray(x, np.float32), np.asarray(wq, np.float32),
                           np.asarray(wk, np.float32), np.asarray(wv, np.float32),
                           np.asarray(wo, np.float32))
    res = bass_utils.run_bass_kernel_spmd(
        nc, in_maps, core_ids=list(range(8)),
        trace=bool(os.environ.get("BASS_TRACE")))
    LAST_RESULTS = res
    acc = res.results[0]["outT"].astype(np.float64)
    for c in range(1, 8):
        acc = acc + res.results[c]["outT"]
    return np.ascontiguousarray(acc.T).astype(np.float32).reshape(1, S, D)


# revision 31
# speedup vs baseline: 1.0902x; 1.0403x over previous
"""Llama-style GQA attention (B=1, S=2048, D=4096, 32 q-heads / 8 kv-heads,
rope, causal) on 8 trn2 NeuronCores, tensor-parallel over heads.

Core c owns q-heads 4c..4c+3 and kv-head c. Activations live in
"transposed" (feature-on-partition, seq-on-free) layout so every matmul
contracts over the partition dim. W_O is row-sharded; each core emits a
partial (D, S) fp16 output and the host sums the 8 partials.

v3 schedule: one dense PE pipeline, software-pipelined at the seq-chunk
level: cp0, rope0, cp1, attn0, rope1, cp2, attn1, ... so the rope DVE
chain for chunk j always executes under the next chunk's projection
matmuls, never on the PE critical path.
 - QKV projection per seq-chunk j: all 6 accumulators (4 q-feature tiles
   + k + v) live in PSUM as three 2-bank tiles; weights resident in SBUF
   loaded with 12 batched descriptors (8 k-tiles per DMA), x streams in
   1MB chunks (DMA issue rate was the v2 bottleneck, not bandwidth).
 - PSUM accumulators evacuate through fast ACT copies into SBUF staging,
   freeing banks for the next chunk within ~4us.
 - Scores for 2 k-tiles share one [128,1024] 2-bank PSUM tile -> single
   ACTIVATE exp (amortizes the 352-cycle ACT overhead).
 - Softmax denominator: ones-column matmuls accumulate next to ctx;
   normalization = gpsimd partition-broadcast + reciprocal_approx_fast
   (51 ULP, ~5x faster than DVE reciprocal) + one in-place multiply,
   all off the PE path.
 - O-projection writes fp16 partials (halves the output DMA).

RoPE trick: wq/wk rows are de-interleaved per head on the host
([0,2,..,126,1,3,..,127]) so the on-device pair (2j, 2j+1) becomes
(j, j+64) — a 64-partition block swap done with partition-offset vector
ops against host-precomputed sign-folded cos/sin tables. The permutation
cancels in Q.K, and V/W_O are untouched by it.

Softmax needs no max-subtraction: scores are bounded by construction
(|s| < ~10 => exp safe in fp32). scoresT layout (k on partitions, q on
free) means P feeds P@V with no transpose.
"""
import os
import numpy as np
import ml_dtypes

S = 2048
D = 4096
HD = 128
CH = 512
KT = 32          # contraction tiles over D
NJ = 4           # seq chunks
SCALE = 1.0 / np.sqrt(128.0)

_cache = {}


def _build():
    import concourse.bacc as bacc
    import concourse.tile as tile
    import concourse.mybir as mybir
    from concourse import bass, bass_isa

    dt = mybir.dt
    nc = bacc.Bacc("TRN2", target_bir_lowering=False, debug=False,
                   enable_asserts=False, num_devices=8)

    def inp(name, shape, d):
        return nc.dram_tensor(name, shape, d, kind="ExternalInput").ap()

    # host-pre-tiled inputs: partition-major so every DMA line is >=1KB
    # contiguous (plain feature-major layouts gather 1KB rows at ~85GB/s;
    # these stream at near-full HBM bandwidth)
    xtld = inp("xtld", (HD, NJ, KT, CH), dt.bfloat16)
    wqtld = inp("wqtld", (HD, KT, 4 * HD), dt.bfloat16)
    wktld = inp("wktld", (HD, KT, HD), dt.bfloat16)
    wvtld = inp("wvtld", (HD, KT, HD), dt.bfloat16)
    wotld = inp("wotld", (HD, 32, 4, HD), dt.bfloat16)
    cosd = inp("cosd", (HD, S), dt.float32)
    sind = inp("sind", (HD, S), dt.float32)
    maskd = inp("maskd", (HD, 4 * CH), dt.bfloat16)
    identd = inp("identd", (HD, HD), dt.bfloat16)
    onesc = inp("onesc", (HD, 1), dt.bfloat16)
    outT = nc.dram_tensor("outT", (D, S), dt.float16, kind="ExternalOutput").ap()

    Exp = mybir.ActivationFunctionType.Exp

    with tile.TileContext(nc) as tc:
        with (
            tc.tile_pool(name="const", bufs=1) as constp,
            tc.tile_pool(name="wres", bufs=1) as wresp,
            tc.tile_pool(name="xs", bufs=8) as xpool,
            tc.tile_pool(name="wo", bufs=6) as wopool,
            tc.tile_pool(name="acts", bufs=1) as actp,
            tc.tile_pool(name="pt", bufs=4) as ptpool,
            tc.tile_pool(name="tmp", bufs=2) as tmpp,
            tc.tile_pool(name="ost", bufs=4) as ostp,
            tc.tile_pool(name="ps", bufs=4, space="PSUM") as psp,
        ):

            # resident weights. Each DMA ring sustains only ~70-90GB/s, so
            # chunk finely (4 k-tiles) and round-robin rings by need-time.
            NC_ = 8
            CHUNKS = [(4 * i, 4) for i in range(NC_)]
            wq8 = [wresp.tile([HD, 4, 4 * HD], dt.bfloat16, tag=f"wq8_{c}",
                              name=f"wq8_{c}") for c in range(NC_)]
            wk8 = [wresp.tile([HD, 4, HD], dt.bfloat16, tag=f"wk8_{c}",
                              name=f"wk8_{c}") for c in range(NC_)]
            wv8 = [wresp.tile([HD, 4, HD], dt.bfloat16, tag=f"wv8_{c}",
                              name=f"wv8_{c}") for c in range(NC_)]

            def load_wc(c, eng):
                k0, nk = CHUNKS[c]
                eng.dma_start(wq8[c][:], wqtld[:, k0:k0 + nk, :])
                eng.dma_start(wk8[c][:], wktld[:, k0:k0 + nk, :])
                eng.dma_start(wv8[c][:], wvtld[:, k0:k0 + nk, :])

            load_wc(0, nc.scalar)
            onesc_t = constp.tile([HD, 1], dt.bfloat16, tag="onesc")
            nc.gpsimd.dma_start(onesc_t[:], onesc[:])
            ident_t = constp.tile([HD, HD], dt.bfloat16, tag="ident")
            nc.gpsimd.dma_start(ident_t[:], identd[:])
            # warm the ACT exp table while startup DMAs run
            dummy = tmpp.tile([HD, 1], dt.float32, tag="dummy", bufs=1)
            nc.scalar.activation(dummy[:], onesc_t[:], Exp)
            for c in range(1, NC_):
                load_wc(c, nc.scalar if c % 2 == 0 else nc.gpsimd)
            cos_t = constp.tile([HD, S], dt.float32, tag="cos")
            nc.scalar.dma_start(cos_t[:], cosd[:])
            sin_t = constp.tile([HD, S], dt.float32, tag="sin")
            nc.gpsimd.dma_start(sin_t[:], sind[:])
            mask_t = constp.tile([HD, 4 * CH], dt.bfloat16, tag="mask")
            nc.gpsimd.dma_start(mask_t[:], maskd[:])

            # persistent activations (feature x seq)
            ktr = actp.tile([HD, S], dt.bfloat16, tag="ktr")
            vbuf = actp.tile([HD, S], dt.bfloat16, tag="vbuf")  # (k 128, kt*128 d)
            ctxs = [actp.tile([HD, S], dt.bfloat16, tag=f"ctx{h}", name=f"ctx{h}")
                    for h in range(4)]

            def rope_into(dst, src, ch):
                """dst (bf16 [128,512]) = src*COS + swap64(src)*SIN at chunk ch"""
                c0 = ch * CH
                t1 = tmpp.tile([HD, CH], dt.float32, tag="r1")
                nc.vector.tensor_mul(t1[:], src, cos_t[:, c0:c0 + CH])
                t2 = tmpp.tile([HD, CH], dt.float32, tag="r2")
                nc.vector.tensor_mul(t2[0:64, :], src[64:128, :], sin_t[64:128, c0:c0 + CH])
                nc.vector.tensor_mul(t2[64:128, :], src[0:64, :], sin_t[0:64, c0:c0 + CH])
                nc.vector.tensor_add(dst, t1[:], t2[:])

            qtr = {}

            def chpass(j, mid=None):
                """project x chunk j; stage to SBUF. rope + v-transpose deferred."""
                qA = psp.tile([HD, 2 * CH], dt.float32, tag="b2", name=f"qA{j}")
                qB = psp.tile([HD, 2 * CH], dt.float32, tag="b2", name=f"qB{j}")
                kvA = psp.tile([HD, 2 * CH], dt.float32, tag="b2", name=f"kv{j}")
                for c, (k0, nk) in enumerate(CHUNKS):
                    x8 = xpool.tile([HD, 4, CH], dt.bfloat16, tag="x8",
                                    name=f"x8_{j}_{c}")
                    if j == 0 or c % 2 == 0:
                        xeng = nc.sync
                    else:  # issued ahead of attn(j-1)'s exps on this queue
                        xeng = nc.scalar
                    xeng.dma_start(x8[:, 0:nk, :],
                                   xtld[:, j, k0:k0 + nk, :])
                    for kk in range(nk):
                        k = k0 + kk
                        st = (k == 0)
                        sp = (k == KT - 1)
                        xs = x8[:, kk, :]
                        nc.tensor.matmul(qA[:, 0:CH], wq8[c][:, kk, 0:HD], xs, start=st, stop=sp)
                        nc.tensor.matmul(qA[:, CH:2 * CH], wq8[c][:, kk, HD:2 * HD], xs, start=st, stop=sp)
                        nc.tensor.matmul(qB[:, 0:CH], wq8[c][:, kk, 2 * HD:3 * HD], xs, start=st, stop=sp)
                        nc.tensor.matmul(qB[:, CH:2 * CH], wq8[c][:, kk, 3 * HD:4 * HD], xs, start=st, stop=sp)
                        nc.tensor.matmul(kvA[:, 0:CH], wk8[c][:, kk, :], xs, start=st, stop=sp)
                        nc.tensor.matmul(kvA[:, CH:2 * CH], wv8[c][:, kk, :], xs, start=st, stop=sp)
                    if c == 1 and mid is not None:
                        mid()  # prev chunk's v-transposes, off the boundary
                # evacuate PSUM accumulators to SBUF staging, split across the
                # two PSUM-capable engines so attn's first exp starts ~2us sooner
                vstage = tmpp.tile([HD, CH], dt.bfloat16, tag="vstage",
                                   name=f"vstage{j}")
                nc.scalar.copy(vstage[:], kvA[:, CH:2 * CH])
                qsB = tmpp.tile([HD, 2 * CH], dt.float32, tag="qsB", bufs=1, name=f"qsB{j}")
                nc.vector.tensor_copy(qsB[:], qB[:])
                kst = tmpp.tile([HD, CH], dt.float32, tag="kst", name=f"kst{j}")
                nc.vector.tensor_copy(kst[:], kvA[:, 0:CH])
                qsA = tmpp.tile([HD, 2 * CH], dt.float32, tag="qsA", bufs=1, name=f"qsA{j}")
                nc.scalar.copy(qsA[:], qA[:])
                return kst, qsA, qsB, vstage

            def vxpose(j, vstage):
                """v: PE-transpose 4x128 into one psum slot, single copy to vbuf"""
                vtp = psp.tile([HD, 4 * CH], dt.bfloat16, tag="b2", name=f"vtp{j}")
                for t in range(4):
                    nc.tensor.transpose(vtp[:, t * HD:(t + 1) * HD],
                                        vstage[:, t * HD:(t + 1) * HD], ident_t[:])
                nc.vector.tensor_copy(vbuf[:, j * CH:(j + 1) * CH], vtp[:, 0:CH])

            def rope_block(j, kst, qsA, qsB, _vstage=None):
                rope_into(ktr[:, j * CH:(j + 1) * CH], kst[:], j)
                for f in range(4):
                    qt = ptpool.tile([HD, CH], dt.bfloat16, tag="qtr", bufs=8,
                                     name=f"qtr{j}_{f}")
                    src = qsA if f < 2 else qsB
                    rope_into(qt[:], src[:, (f % 2) * CH:(f % 2 + 1) * CH], j)
                    qtr[(j, f)] = qt

            def attn(j):
                for h in range(4):
                    ng = 2 * (j + 1)
                    ctxden = psp.tile([HD, 2 * CH], dt.float32, tag="b2",
                                      name=f"cd{j}_{h}")
                    pts = []

                    def score_exp(g):
                        sg = psp.tile([HD, 2 * CH], dt.float32, tag="b2",
                                      name=f"sg{j}_{h}_{g}")
                        pt = ptpool.tile([HD, 2 * CH], dt.bfloat16, tag="pt",
                                         bufs=5, name=f"pt{j}_{h}_{g}")
                        if g < 2 * j:  # full group: batched exp, no mask
                            nc.tensor.matmul(sg[:, 0:CH],
                                             ktr[:, (2 * g) * HD:(2 * g + 1) * HD],
                                             qtr[(j, h)][:], start=True, stop=True)
                            nc.tensor.matmul(sg[:, CH:2 * CH],
                                             ktr[:, (2 * g + 1) * HD:(2 * g + 2) * HD],
                                             qtr[(j, h)][:], start=True, stop=True)
                            nc.scalar.activation(pt[:], sg[:], Exp, scale=SCALE)
                        else:  # diagonal: ragged live region + [128,128] tri mask
                            for half in range(2):
                                kt = 2 * g + half
                                q0 = (kt - 4 * j) * HD
                                c0, c1 = half * CH + q0, (half + 1) * CH
                                nc.tensor.matmul(sg[:, c0:c1],
                                                 ktr[:, kt * HD:(kt + 1) * HD],
                                                 qtr[(j, h)][:, q0:CH],
                                                 start=True, stop=True)
                                nc.scalar.activation(pt[:, c0:c1], sg[:, c0:c1],
                                                     Exp, scale=SCALE)
                                nc.vector.tensor_mul(pt[:, c0:c0 + HD],
                                                     pt[:, c0:c0 + HD],
                                                     mask_t[:, 0:HD])
                        pts.append(pt)

                    def pv_den(g):
                        pt = pts[g]
                        halves = []
                        for half in range(2):
                            kt = 2 * g + half
                            q0 = max(0, (kt - 4 * j)) * HD
                            halves.append((half, kt, q0,
                                           g == 0 and half == 0,
                                           g == ng - 1 and half == 1))
                        for half, kt, q0, st, sp in halves:
                            nc.tensor.matmul(ctxden[:, q0:CH],
                                             vbuf[:, kt * HD:(kt + 1) * HD],
                                             pt[:, half * CH + q0:(half + 1) * CH],
                                             start=st, stop=sp)
                        for half, kt, q0, st, sp in halves:
                            nc.tensor.matmul(ctxden[0:1, CH + q0:2 * CH],
                                             onesc_t[:],
                                             pt[:, half * CH + q0:(half + 1) * CH],
                                             start=st, stop=sp)

                    LAG = 3
                    for gi in range(ng + LAG):
                        if gi < ng:
                            score_exp(gi)
                        if gi >= LAG:
                            pv_den(gi - LAG)
                    # evacuate raw ctx via ACT (idle here; frees bank fast);
                    # den row -> broadcast -> one partition-parallel
                    # approx-reciprocal -> in-place scale, all off the PE path
                    nc.scalar.copy(ctxs[h][:, j * CH:(j + 1) * CH],
                                   ctxden[:, 0:CH])
                    densb = tmpp.tile([1, CH], dt.float32, tag="densb",
                                      name=f"densb{j}_{h}")
                    nc.vector.tensor_copy(densb[:], ctxden[0:1, CH:2 * CH])
                    bcs = tmpp.tile([HD, CH], dt.float32, tag="bcs",
                                    name=f"bcs{j}_{h}")
                    nc.gpsimd.partition_broadcast(bcs[:], densb[:], channels=HD)
                    rcp = tmpp.tile([HD, CH], dt.float32, tag="rcp",
                                    name=f"rcp{j}_{h}")
                    nc.vector.reciprocal_approx_fast(rcp[:], bcs[:])
                    nc.vector.tensor_mul(ctxs[h][:, j * CH:(j + 1) * CH],
                                         ctxs[h][:, j * CH:(j + 1) * CH], rcp[:])

            # software pipeline: rope(j) always runs under cp(j+1)/attn PE work;
            # vxpose(j) is emitted inside cp(j+1)'s k-loop, off the boundary
            staged = {0: chpass(0)}
            rope_block(0, *staged[0])
            for j in range(NJ):
                if j + 1 < NJ:
                    staged[j + 1] = chpass(
                        j + 1, mid=lambda vj=j: vxpose(vj, staged[vj][3]))
                attn(j)
                if j + 1 < NJ:
                    rope_block(j + 1, *staged[j + 1])
                if j == NJ - 2:
                    vxpose(NJ - 1, staged[NJ - 1][3])
                    # prefetch the first O-proj weights ahead of attn(3)'s
                    # partition_all_reduces on the gpsimd queue
                    wo_pre = []
                    for of in range(4):
                        w = wopool.tile([HD, 4, HD], dt.bfloat16, tag="wo",
                                        name=f"wopre{of}")
                        nc.gpsimd.dma_start(w[:], wotld[:, of, :, :])
                        wo_pre.append(w)

            # ---- O projection (row-sharded W_O -> partial fp16 outT) ----
            for of in range(32):
                oA = psp.tile([HD, 2 * CH], dt.float32, tag="b2", name=f"oA{of}")
                oB = psp.tile([HD, 2 * CH], dt.float32, tag="b2", name=f"oB{of}")
                if of < 4:
                    wot8 = wo_pre[of]
                else:
                    wot8 = wopool.tile([HD, 4, HD], dt.bfloat16, tag="wo")
                    nc.gpsimd.dma_start(wot8[:], wotld[:, of, :, :])
                for cf in range(4):
                    st = (cf == 0)
                    sp = (cf == 3)
                    for ch in range(4):
                        dstp = oA if ch < 2 else oB
                        nc.tensor.matmul(dstp[:, (ch % 2) * CH:(ch % 2 + 1) * CH],
                                         wot8[:, cf, :],
                                         ctxs[cf][:, ch * CH:(ch + 1) * CH],
                                         start=st, stop=sp)
                ost1 = ostp.tile([HD, 2 * CH], dt.float16, tag="ostA")
                ost2 = ostp.tile([HD, 2 * CH], dt.float16, tag="ostB")
                if of < 31:
                    nc.vector.tensor_copy(ost1[:], oA[:])
                    nc.scalar.copy(ost2[:], oB[:])
                    nc.sync.dma_start(outT[of * HD:(of + 1) * HD, 0:2 * CH], ost1[:])
                    nc.sync.dma_start(outT[of * HD:(of + 1) * HD, 2 * CH:4 * CH], ost2[:])
                else:  # drain the tail fast: split work across engines/queues
                    nc.vector.tensor_copy(ost1[:, 0:CH], oA[:, 0:CH])
                    nc.scalar.copy(ost2[:, 0:CH], oB[:, 0:CH])
                    nc.sync.dma_start(outT[of * HD:(of + 1) * HD, 0:CH], ost1[:, 0:CH])
                    nc.gpsimd.dma_start(outT[of * HD:(of + 1) * HD, 2 * CH:3 * CH], ost2[:, 0:CH])
                    nc.vector.tensor_copy(ost1[:, CH:2 * CH], oA[:, CH:2 * CH])
                    nc.scalar.copy(ost2[:, CH:2 * CH], oB[:, CH:2 * CH])
                    nc.sync.dma_start(outT[of * HD:(of + 1) * HD, CH:2 * CH], ost1[:, CH:2 * CH])
                    nc.gpsimd.dma_start(outT[of * HD:(of + 1) * HD, 3 * CH:4 * CH], ost2[:, CH:2 * CH])

    nc.compile()
    return nc


def _host_inputs(x, wq, wk, wv, wo):
    bf16 = ml_dtypes.bfloat16
    perm = np.concatenate([np.arange(0, 128, 2), np.arange(1, 128, 2)])
    half = 64
    inv = 1.0 / (10000.0 ** (np.arange(half) / half))
    ang = np.arange(S)[:, None] * inv[None, :]
    cosd = np.ascontiguousarray(
        np.concatenate([np.cos(ang).T, np.cos(ang).T], 0)).astype(np.float32)
    sind = np.ascontiguousarray(
        np.concatenate([np.sin(ang).T, -np.sin(ang).T], 0)).astype(np.float32)
    maskd = np.zeros((HD, 4 * CH), np.float32)
    for m in range(4):
        kl = np.arange(HD)[:, None]
        maskd[:, m * CH:(m + 1) * CH] = (np.arange(CH)[None, :] >= HD * m + kl)
    maskd = maskd.astype(bf16)
    identd = np.eye(HD, dtype=bf16)
    onescol = np.ones((HD, 1), bf16)
    xTb = np.ascontiguousarray(x[0].T).astype(bf16)

    def tile_pkn(wT):  # (D, n) feature-major -> (128, 32k, n) partition-major
        n = wT.shape[1]
        return np.ascontiguousarray(
            wT.reshape(KT, HD, n).transpose(1, 0, 2)).astype(bf16)

    xtld = np.ascontiguousarray(
        xTb.reshape(KT, HD, NJ, CH).transpose(1, 2, 0, 3))  # (128, j, k, n)

    in_maps = []
    for c in range(8):
        qrows = slice(512 * c, 512 * (c + 1))
        wq_c = wq[qrows].reshape(4, HD, D)[:, perm].reshape(512, D)
        wk_c = wk[HD * c:HD * (c + 1)][perm]
        wv_c = wv[HD * c:HD * (c + 1)]
        wo_c = np.ascontiguousarray(wo[:, qrows].T)  # (512, 4096)
        wotld = np.ascontiguousarray(
            wo_c.reshape(4, HD, 32, HD).transpose(1, 2, 0, 3)).astype(bf16)
        in_maps.append({
            "xtld": xtld,
            "wqtld": tile_pkn(np.ascontiguousarray(wq_c.T)),
            "wktld": tile_pkn(np.ascontiguousarray(wk_c.T)),
            "wvtld": tile_pkn(np.ascontiguousarray(wv_c.T)),
            "wotld": wotld,
            "cosd": cosd, "sind": sind, "maskd": maskd, "identd": identd,
            "onesc": onescol,
        })
    return in_maps


LAST_RESULTS = None


def kernel(x, wq, wk, wv, wo, attn_mask):
    global LAST_RESULTS
    from concourse import bass_utils
    if "nc" not in _cache:
        _cache["nc"] = _build()
    nc = _cache["nc"]
    in_maps = _host_inputs(np.asarray(x, np.float32), np.asarray(wq, np.float32),
                           np.asarray(wk, np.float32), np.asarray(wv, np.float32),
                           np.asarray(wo, np.float32))
    res = bass_utils.run_bass_kernel_spmd(
        nc, in_maps, core_ids=list(range(8)),
        trace=bool(os.environ.get("BASS_TRACE")))
    LAST_RESULTS = res
    acc = res.results[0]["outT"].astype(np.float64)
    for c in range(1, 8):
        acc = acc + res.results[c]["outT"]
    return np.ascontiguousarray(acc.T).astype(np.float32).reshape(1, S, D)
